# revision 28
# baseline (speedup 1.0000x reference)
"""Masked 5x5 conv (PixelCNN 'A' mask) on 8 Trainium2 NeuronCores.

Problem (hardcoded): x[4,192,128,128] f32, weight[384,192,5,5] f32,
bias[384] f32, mask[4,1,128,128] i32.
out = where(window_any(mask), conv(x, weight*maskA) + bias, 0).

The 'A' causal mask keeps 12 of 25 taps: rows kh=0,1 fully, row kh=2 only
kw=0,1 -- i.e. every tap reads the current output row or rows above it.

Sharding: core c = (batch b = c//2, row-half = c%2). Each core computes one
batch's 64 output rows for all 384 out channels (3 M=128 chunks).

Per output tile [128 cout, 4 rows x 128 cols = 512] we accumulate 16
matmuls into one PSUM bank (contraction 12 taps x 192 cin = 2304):
  - 8 bf16 taps x ci[0:128]            (tile xa)
  - 2 fp8e4 DoubleRow slots x ci[0:128]: taps (0,j)+(1,j) for j=0,1
    packed as K=256 (two k-tiles) via a [p, t:WP, r:WP, c:1] strided AP
    on the fp8 copy of xa -- each runs in the time of ONE bf16 matmul.
  - 5 bf16 tap-PAIRS x ci[128:192]     (tile xb: lower 64 partitions =
    ci[128:192] data, upper 64 = same data shifted 1 col)
  - 1 bf16 tap-pair (0,4)+(1,4) x ci[128:192] (tile xc: upper shifted 1 row)
All weights are pre-scaled x256 on host (exact in bf16; lifts the fp8
weights out of the e4m3 denormal range). PSUM holds 256*conv; the DVE
epilogue is a plain f32->bf16 copy, and the host applies /256 + bias and
the window-any(mask) zeroing in f32 during assembly.
"""

import numpy as np
import ml_dtypes

import bass_rust
import concourse.bass as bass
from concourse import mybir
from concourse.bass_utils import run_bass_kernel_spmd

B, CIN, COUT, H, W = 4, 192, 384, 128, 128
KH = KW = 5
PAD = 2
NCORES = 8
HHALF = 64          # output rows per core
NROWS = HHALF + 2   # input rows staged per core (2 above)
WP = W + 4          # padded width
FLAT = NROWS * WP   # 66*132 = 8712
RB = 4              # output rows per block
NBLK = HHALF // RB  # 16 blocks
NFREE = RB * W      # 512 = one PSUM bank of fp32
WSCALE = 256.0      # weight pre-scale (power of 2; undone on host)

# bf16 xa taps of the 'A' mask, ci[0:128] (slots 0..7)
TAPS_BF = [(0, 2), (0, 3), (0, 4), (1, 2), (1, 3), (1, 4), (2, 0), (2, 1)]
# fp8 DoubleRow slots: vertical tap pairs (0,j)+(1,j), ci[0:128]
DR_COLS = [0, 1]
# ci[128:192] handled as bf16 pairs packed into K=128 matmuls.
PAIRS_XB = [((0, 0), (0, 1)), ((0, 2), (0, 3)),
            ((1, 0), (1, 1)), ((1, 2), (1, 3)), ((2, 0), (2, 1))]
PAIR_XC = ((0, 4), (1, 4))
NSLOT = 14          # bf16 weight slots per m-chunk: 8 xa + 5 xb + 1 xc

BF16 = ml_dtypes.bfloat16
FP8 = ml_dtypes.float8_e4m3
DRM = mybir.MatmulPerfMode.DoubleRow


def _build_program():
    """Raw Bass (no Tile): this walrus build rejects instructions carrying
    more than ~1 embedded sync wait, so all synchronization is standalone
    wait_ge instructions with manually-managed semaphores.

    Schedule (per core, ~180us):
      - The framework preamble holds every engine until ~8us; the first
        DMA cannot issue before ~7.5us and each dma_start costs ~0.6us of
        Sync issue time, so wave 1 is kept to 7 streams.
      - PE pre-warm: 11 dummy matmuls (~4.7us at the cold clock) end
        about when wave 1 lands, flipping the HAM clock gate to full
        speed with no idle gap (a >2us PE gap resets the clock).
      - No phases: tile k is a contiguous 16-matmul group gated on
        interleaved row-chunks of all four x slabs, so the PE never
        waits mid-tile and the clock never drops.
      - DVE drains each PSUM bank with a plain f32->bf16 copy; outputs
        stream out in 2-tile chunks with a split final chunk."""
    nc = bass.Bass()
    bf = mybir.dt.bfloat16
    f8 = mybir.dt.float8e4

    xa_d = nc.dram_tensor("xa", [128, FLAT], bf, kind="ExternalInput")
    x8_d = nc.dram_tensor("x8", [128, FLAT], f8, kind="ExternalInput")
    xb_d = nc.dram_tensor("xb", [128, FLAT], bf, kind="ExternalInput")
    xc_d = nc.dram_tensor("xc", [128, FLAT], bf, kind="ExternalInput")
    wt_d = nc.dram_tensor("wt", [128, 3 * NSLOT * 128], bf, kind="ExternalInput")
    w8_d = nc.dram_tensor("w8", [128, 3 * 2 * 256], f8, kind="ExternalInput")
    out_d = nc.dram_tensor("out", [128, 3 * HHALF * W], bf, kind="ExternalOutput")

    NPS = 8           # psum banks in rotation
    OCH = 4           # out-DMA granularity: blocks per chunk
    NT = 3 * NBLK     # 48 tiles
    WTM = NSLOT * 128  # wt cols per m-chunk
    # x row-chunk upper bounds (exclusive); tile k reads x rows <= 4*(k%16)+5,
    # so chunk c is first needed by tile FIRST_TILE[c]
    CH = [6, 14, 26, 38, 50, 62, 66]
    FIRST_TILE = [0, 1, 3, 6, 9, 12, 15]

    from contextlib import ExitStack
    with ExitStack() as ctx:
        xa_t = ctx.enter_context(nc.sbuf_tensor([128, FLAT], bf))
        x8_t = ctx.enter_context(nc.sbuf_tensor([128, FLAT], f8))
        xb_t = ctx.enter_context(nc.sbuf_tensor([128, FLAT], bf))
        xc_t = ctx.enter_context(nc.sbuf_tensor([128, FLAT], bf))
        wt_t = ctx.enter_context(nc.sbuf_tensor([128, 3 * NSLOT * 128], bf))
        w8_t = ctx.enter_context(nc.sbuf_tensor([128, 3 * 2 * 256], f8))
        st_t = ctx.enter_context(nc.sbuf_tensor([128, 3 * HHALF * W], bf))
        ps_t = ctx.enter_context(nc.psum_tensor([128, NPS * NFREE], mybir.dt.float32))
        dwt = ctx.enter_context(nc.semaphore("dwt"))
        dxa0 = ctx.enter_context(nc.semaphore("dxa0"))
        dxb0 = ctx.enter_context(nc.semaphore("dxb0"))
        dx = ctx.enter_context(nc.semaphore("dx"))
        dwtr = ctx.enter_context(nc.semaphore("dwtr"))
        pes = ctx.enter_context(nc.semaphore("pes"))
        dve = ctx.enter_context(nc.semaphore("dve"))
        dout = ctx.enter_context(nc.semaphore("dout"))
        warm = ctx.enter_context(nc.semaphore("warm"))
        block = ctx.enter_context(nc.Block())
        xa_v = xa_t[:].rearrange("p (r c) -> p r c", c=WP)
        xb_v = xb_t[:].rearrange("p (r c) -> p r c", c=WP)
        xc_v = xc_t[:].rearrange("p (r c) -> p r c", c=WP)

        def wt_ap(m, s):
            lo = (m * NSLOT + s) * 128
            return wt_t[:, lo:lo + 128]

        def w8_ap(m, d):
            lo = (m * 2 + d) * 256
            return w8_t[:, lo:lo + 256].rearrange("p (t q) -> p t q", t=2)

        def dr_mv(j0, j, h0=0, rsz=RB):
            # moving AP [p, t(2):WP, r(rsz):WP, c(W):1] at row j0+h0, col j
            return bass_rust.AP(
                x8_t[:].tensor, (j0 + h0) * WP + j,
                bass_rust.VecI64Pair(
                    [[FLAT, 128], [WP, 2], [WP, rsz], [1, W]]))

        QBANK = [7, 4, 5, 6]   # final-tile quarter -> psum bank

        def emit_tile_a(k, h0=0, rsz=RB, qb=None):
            # part 1: the 10 ci[0:128] slots (8 bf16 xa + 2 fp8 DR)
            m, blk = divmod(k, NBLK)
            j0 = blk * RB
            if qb is None:
                ps = ps_t[:, (k % NPS) * NFREE:(k % NPS + 1) * NFREE]
            else:
                ps = ps_t[:, qb * NFREE:qb * NFREE + rsz * W]
            for s, (kh, kw) in enumerate(TAPS_BF):
                nc.tensor.matmul(
                    ps, wt_ap(m, s),
                    xa_v[:, j0 + h0 + kh: j0 + h0 + kh + rsz, kw: kw + W],
                    start=(s == 0), stop=False)
            for d in DR_COLS:
                nc.tensor.matmul(ps, w8_ap(m, d), dr_mv(j0, d, h0=h0, rsz=rsz),
                                 start=False, stop=False, perf_mode=DRM)

        def emit_tile_b(k, h0=0, rsz=RB, qb=None):
            # part 2: the 6 ci[128:192] pair slots
            m, blk = divmod(k, NBLK)
            j0 = blk * RB
            if qb is None:
                ps = ps_t[:, (k % NPS) * NFREE:(k % NPS + 1) * NFREE]
            else:
                ps = ps_t[:, qb * NFREE:qb * NFREE + rsz * W]
            for i, (ta, _tb) in enumerate(PAIRS_XB):
                nc.tensor.matmul(
                    ps, wt_ap(m, 8 + i),
                    xb_v[:, j0 + h0 + ta[0]: j0 + h0 + ta[0] + rsz,
                         ta[1]: ta[1] + W],
                    start=False, stop=False)
            mm = nc.tensor.matmul(
                ps, wt_ap(m, 13),
                xc_v[:, j0 + h0 + PAIR_XC[0][0]: j0 + h0 + PAIR_XC[0][0] + rsz,
                     PAIR_XC[0][1]: PAIR_XC[0][1] + W],
                start=False, stop=True)
            mm.then_inc(pes, 1)

        def emit_tile(k):
            emit_tile_a(k)
            emit_tile_b(k)

        @block.sync
        def _(sync):
            def split2(dst, src, lo, hi, sem):
                mid = ((lo + hi) // 2 // 4) * 4
                sync.dma_start(dst[:, lo:mid], src[:, lo:mid]).then_inc(sem, 16)
                sync.dma_start(dst[:, mid:hi], src[:, mid:hi]).then_inc(sem, 16)

            def xchunk(c):
                lo = 0 if c == 0 else CH[c - 1] * WP
                hi = CH[c] * WP
                for dst, src in ((xa_t, xa_d), (xb_t, xb_d), (xc_t, xc_d),
                                 (x8_t, x8_d)):
                    sync.dma_start(dst[:, lo:hi], src[:, lo:hi]).then_inc(dx, 16)

            # wave 1, in issue-priority order (each dma_start costs ~0.6us
            # of Sync time, so order = priority): tile 0's first 10 matmuls
            # need only wt slots 0..7 (the first two thirds of the m0
            # chunk), w8-m0, and xa/x8 rows [0,6); its last 6 matmuls add
            # the pair-slot weights (third wt chunk) and xb/xc rows.
            W3 = WTM // 3 // 4 * 4
            c0 = CH[0] * WP
            sync.dma_start(wt_t[:, 0:W3], wt_d[:, 0:W3]).then_inc(dwt, 16)
            sync.dma_start(wt_t[:, W3:2 * W3], wt_d[:, W3:2 * W3]).then_inc(dwt, 16)
            split2(xa_t, xa_d, 0, c0, dxa0)
            sync.dma_start(x8_t[:, 0:c0], x8_d[:, 0:c0]).then_inc(dxa0, 16)
            sync.dma_start(w8_t[:, 0:512], w8_d[:, 0:512]).then_inc(dwt, 16)
            sync.dma_start(wt_t[:, 2 * W3:WTM], wt_d[:, 2 * W3:WTM]).then_inc(dxb0, 16)
            sync.dma_start(xb_t[:, 0:c0], xb_d[:, 0:c0]).then_inc(dxb0, 16)
            sync.dma_start(xc_t[:, 0:c0], xc_d[:, 0:c0]).then_inc(dxb0, 16)
            xchunk(1)
            # keep ~2 chunks in flight: issue chunk c+2 once chunk c landed
            sync.wait_ge(dxb0, 48)
            xchunk(2)
            for c in range(3, len(CH)):
                sync.wait_ge(dx, 64 * (c - 2))
                xchunk(c)
            # m1/m2 weights (needed from tile 16)
            split2(wt_t, wt_d, WTM, 2 * WTM, dwtr)
            split2(wt_t, wt_d, 2 * WTM, 3 * WTM, dwtr)
            sync.dma_start(w8_t[:, 512:1536], w8_d[:, 512:1536]).then_inc(dwtr, 16)
            # output: tiles 0..43 in chunks of OCH=4 (fewer, bigger DMAs
            # contend less with the PE's SBUF reads), then a tapered tail:
            # tiles 44-45 paired, 46 alone, 47 as four quarter-DMAs
            for c in range(11):
                lo, hi = c * OCH * NFREE, (c + 1) * OCH * NFREE
                sync.wait_ge(dve, OCH * (c + 1))
                sync.dma_start(out_d[:, lo:hi], st_t[:, lo:hi]).then_inc(dout, 16)
            sync.wait_ge(dve, 46)
            sync.dma_start(out_d[:, 44 * NFREE:46 * NFREE],
                           st_t[:, 44 * NFREE:46 * NFREE]).then_inc(dout, 16)
            sync.wait_ge(dve, 47)
            sync.dma_start(out_d[:, 46 * NFREE:47 * NFREE],
                           st_t[:, 46 * NFREE:47 * NFREE]).then_inc(dout, 16)
            QF = NFREE // 4
            base = 47 * NFREE
            for q in range(4):
                sync.wait_ge(dve, NT + q)
                sync.dma_start(out_d[:, base + q * QF:base + (q + 1) * QF],
                               st_t[:, base + q * QF:base + (q + 1) * QF]
                               ).then_inc(dout, 16)
            sync.wait_ge(dout, 16 * 17)

        @block.tensor
        def _(tensor):
            # pre-warm the PE HAM clock gate during the initial DMA wait.
            # The dummies must be FULL-K (128 partitions): K=1 dummies draw
            # no array power and never flip the clock -- the first ~7
            # full-K matmuls run at ~427ns (low pstate), then ~216ns. 17
            # of them end about when wave 1 lands; any idle gap >~1us
            # drops the clock again. st_t garbage is fine: bank 7's first
            # real user (tile 7) starts its group with start=True.
            tensor.wait_ge(warm, 1)
            for _ in range(18):
                nc.tensor.matmul(
                    ps_t[:, 7 * NFREE:8 * NFREE],
                    st_t[:, 0:128],
                    st_t[:, 0:NFREE],
                    start=True,
                    stop=True,
                )
            # tile 0 split-phase: its first 10 matmuls need only wt/w8 +
            # xa/x8 rows [0,6), which are DMA-issued ahead of xb/xc -- the
            # PE starts ~1.5us earlier and the clock never drops.
            tensor.wait_ge(dwt, 48)
            tensor.wait_ge(dxa0, 48)
            emit_tile_a(0)
            tensor.wait_ge(dxb0, 48)
            emit_tile_b(0)
            gates = {t: 64 * c for c, t in enumerate(FIRST_TILE) if c > 0}
            for k in range(1, NT - 1):
                if k in gates:
                    tensor.wait_ge(dx, gates[k])
                if k == 16:
                    tensor.wait_ge(dwtr, 80)
                # one bank-reuse wait covers 4 tiles: tiles k..k+3 need at
                # most dve >= k+3-(NPS-1), and DVE lags PE by well under
                # the 3-tile slack this leaves.
                if k >= NPS and (k - NPS) % 4 == 0:
                    tensor.wait_ge(dve, min(k + 3, NT - 1) - NPS + 1)
                emit_tile(k)
            # final tile split into four 1-row quarters in four different
            # banks: each quarter's epilogue+out-DMA overlaps the next
            # quarter's matmuls, and the last DMA is only 32KB
            for q in range(4):
                # QBANK[q] is free once its previous tile's copy landed:
                # bank 7 <- tile 39 (dve>=40, covered by the k=44 wait),
                # bank 4/5/6 <- tiles 44/45/46 (dve >= 45/46/47)
                if q > 0:
                    tensor.wait_ge(dve, NT - 4 + q)
                emit_tile_a(NT - 1, h0=q, rsz=1, qb=QBANK[q])
                emit_tile_b(NT - 1, h0=q, rsz=1, qb=QBANK[q])

        @block.vector
        def _(vector):
            nc.vector.memset(st_t[0:1, 0:NFREE], 0.0).then_inc(warm, 1)
            for k in range(NT - 1):
                ps = ps_t[:, (k % NPS) * NFREE:(k % NPS + 1) * NFREE]
                vector.wait_ge(pes, k + 1)
                nc.vector.tensor_copy(
                    st_t[:, k * NFREE:(k + 1) * NFREE], ps).then_inc(dve, 1)
            # final tile: four quarter-width copies matching the split groups
            k = NT - 1
            QF = NFREE // 4
            for q in range(4):
                ps_q = ps_t[:, QBANK[q] * NFREE:QBANK[q] * NFREE + QF]
                vector.wait_ge(pes, k + 1 + q)
                nc.vector.tensor_copy(
                    st_t[:, k * NFREE + q * QF:k * NFREE + (q + 1) * QF],
                    ps_q).then_inc(dve, 1)
    return nc


def _causal_mask():
    m = np.ones((KH, KW), dtype=np.float32)
    m[KH // 2, KW // 2:] = 0.0
    m[KH // 2 + 1:, :] = 0.0
    return m


def _prepare_in_maps(x, weight, bias, mask):
    # window-any of mask -> valid [B, H, W]
    ind = (np.asarray(mask)[:, 0] != 0)
    indp = np.zeros((B, H + 2 * PAD, W + 2 * PAD), dtype=bool)
    indp[:, PAD:PAD + H, PAD:PAD + W] = ind
    valid = np.zeros((B, H, W), dtype=bool)
    for dh in range(KH):
        for dw in range(KW):
            valid |= indp[:, dh:dh + H, dw:dw + W]

    w_sc = np.asarray(weight, dtype=np.float32) * _causal_mask()[None, None] * WSCALE
    w_bf = w_sc.astype(BF16)
    w_f8 = w_sc.astype(FP8)

    # bf16 weights: m-major image [128 K, 3 m-chunks x 14 slots x 128 couts]
    wt = np.zeros((3, NSLOT, 128, 128), dtype=BF16)
    for m in range(3):
        cl, ch = m * 128, (m + 1) * 128
        for s, (kh, kw) in enumerate(TAPS_BF):
            wt[m, s] = w_bf[cl:ch, 0:128, kh, kw].T
        for i, (ta, tb) in enumerate(PAIRS_XB):
            wt[m, 8 + i, 0:64] = w_bf[cl:ch, 128:192, ta[0], ta[1]].T
            wt[m, 8 + i, 64:128] = w_bf[cl:ch, 128:192, tb[0], tb[1]].T
        ta, tb = PAIR_XC
        wt[m, 13, 0:64] = w_bf[cl:ch, 128:192, ta[0], ta[1]].T
        wt[m, 13, 64:128] = w_bf[cl:ch, 128:192, tb[0], tb[1]].T
    wt_sb = np.ascontiguousarray(wt.transpose(2, 0, 1, 3)).reshape(128, 3 * NSLOT * 128)

    # fp8 DR weights: [128 K, 3 m x 2 slots x 2 t x 128 couts], t = kh
    w8 = np.zeros((3, 2, 2, 128, 128), dtype=FP8)
    for m in range(3):
        cl, ch = m * 128, (m + 1) * 128
        for d in DR_COLS:
            for t in range(2):
                w8[m, d, t] = w_f8[cl:ch, 0:128, t, d].T
    w8_sb = np.ascontiguousarray(w8.transpose(3, 0, 1, 2, 4)).reshape(128, 3 * 2 * 256)

    x_f32 = np.asarray(x, dtype=np.float32)
    x_bf = x_f32.astype(BF16)
    x_f8 = x_f32.astype(FP8)

    in_maps = []
    for c in range(NCORES):
        b, half = c // 2, c % 2
        r0 = half * HHALF
        lo = r0 - PAD
        src_lo = max(lo, 0)

        xp = np.zeros((128, NROWS, WP), dtype=BF16)
        xp[:, src_lo - lo:, PAD:PAD + W] = x_bf[b, 0:128, src_lo:r0 + HHALF, :]
        xp8 = np.zeros((128, NROWS, WP), dtype=FP8)
        xp8[:, src_lo - lo:, PAD:PAD + W] = x_f8[b, 0:128, src_lo:r0 + HHALF, :]

        x2p = np.zeros((64, NROWS, WP), dtype=BF16)
        x2p[:, src_lo - lo:, PAD:PAD + W] = x_bf[b, 128:192, src_lo:r0 + HHALF, :]
        x2 = x2p.reshape(64, FLAT)
        sh1 = np.zeros_like(x2)
        sh1[:, :-1] = x2[:, 1:]
        shr = np.zeros_like(x2)
        shr[:, :-WP] = x2[:, WP:]
        in_maps.append({
            "xa": np.ascontiguousarray(xp.reshape(128, FLAT)),
            "x8": np.ascontiguousarray(xp8.reshape(128, FLAT)),
            "xb": np.ascontiguousarray(np.concatenate([x2, sh1], axis=0)),
            "xc": np.ascontiguousarray(np.concatenate([x2, shr], axis=0)),
            "wt": wt_sb,
            "w8": w8_sb,
        })
    return in_maps, valid


def _assemble(results, valid, bias):
    bias_f = np.asarray(bias, dtype=np.float32)
    out_full = np.empty((B, COUT, H, W), dtype=np.float32)
    inv = np.float32(1.0 / WSCALE)
    for c in range(NCORES):
        b, half = c // 2, c % 2
        o = np.asarray(results[c]["out"]).astype(np.float32)
        o4 = o.reshape(128, 3, HHALF, W).transpose(1, 0, 2, 3).reshape(COUT, HHALF, W)
        o4 = o4 * inv + bias_f[:, None, None]
        v = valid[b, half * HHALF:(half + 1) * HHALF, :]
        out_full[b, :, half * HHALF:(half + 1) * HHALF, :] = np.where(v[None], o4, 0.0)
    return out_full


def kernel(x, weight, bias, mask, _trace=False):
    in_maps, valid = _prepare_in_maps(x, weight, bias, mask)
    nc = _build_program()
    res = run_bass_kernel_spmd(nc, in_maps, core_ids=list(range(NCORES)),
                               trace=_trace)
    out = _assemble(res.results, valid, bias)
    if _trace:
        return out, res
    return out


# revision 31
# speedup vs baseline: 1.0047x; 1.0047x over previous
"""Masked 5x5 conv (PixelCNN 'A' mask) on 8 Trainium2 NeuronCores.

Problem (hardcoded): x[4,192,128,128] f32, weight[384,192,5,5] f32,
bias[384] f32, mask[4,1,128,128] i32.
out = where(window_any(mask), conv(x, weight*maskA) + bias, 0).

The 'A' causal mask keeps 12 of 25 taps: rows kh=0,1 fully, row kh=2 only
kw=0,1 -- i.e. every tap reads the current output row or rows above it.

Sharding: core c = (batch b = c//2, row-half = c%2). Each core computes one
batch's 64 output rows for all 384 out channels (3 M=128 chunks).

Per output tile [128 cout, 4 rows x 128 cols = 512] we accumulate 16
matmuls into one PSUM bank (contraction 12 taps x 192 cin = 2304):
  - 8 bf16 taps x ci[0:128]            (tile xa)
  - 2 fp8e4 DoubleRow slots x ci[0:128]: taps (0,j)+(1,j) for j=0,1
    packed as K=256 (two k-tiles) via a [p, t:WP, r:WP, c:1] strided AP
    on the fp8 copy of xa -- each runs in the time of ONE bf16 matmul.
  - 5 bf16 tap-PAIRS x ci[128:192]     (tile xb: lower 64 partitions =
    ci[128:192] data, upper 64 = same data shifted 1 col)
  - 1 bf16 tap-pair (0,4)+(1,4) x ci[128:192] (tile xc: upper shifted 1 row)
All weights are pre-scaled x256 on host (exact in bf16; lifts the fp8
weights out of the e4m3 denormal range). PSUM holds 256*conv; the DVE
epilogue is a plain f32->bf16 copy, and the host applies /256 + bias and
the window-any(mask) zeroing in f32 during assembly.
"""

import numpy as np
import ml_dtypes

import bass_rust
import concourse.bass as bass
from concourse import mybir
from concourse.bass_utils import run_bass_kernel_spmd

B, CIN, COUT, H, W = 4, 192, 384, 128, 128
KH = KW = 5
PAD = 2
NCORES = 8
HHALF = 64          # output rows per core
NROWS = HHALF + 2   # input rows staged per core (2 above)
WP = W + 4          # padded width
FLAT = NROWS * WP   # 66*132 = 8712
RB = 4              # output rows per block
NBLK = HHALF // RB  # 16 blocks
NFREE = RB * W      # 512 = one PSUM bank of fp32
WSCALE = 256.0      # weight pre-scale (power of 2; undone on host)

# bf16 xa taps of the 'A' mask, ci[0:128] (slots 0..7)
TAPS_BF = [(0, 2), (0, 3), (0, 4), (1, 2), (1, 3), (1, 4), (2, 0), (2, 1)]
# fp8 DoubleRow slots: vertical tap pairs (0,j)+(1,j), ci[0:128]
DR_COLS = [0, 1]
# ci[128:192] handled as bf16 pairs packed into K=128 matmuls.
PAIRS_XB = [((0, 0), (0, 1)), ((0, 2), (0, 3)),
            ((1, 0), (1, 1)), ((1, 2), (1, 3)), ((2, 0), (2, 1))]
PAIR_XC = ((0, 4), (1, 4))
NSLOT = 14          # bf16 weight slots per m-chunk: 8 xa + 5 xb + 1 xc

BF16 = ml_dtypes.bfloat16
FP8 = ml_dtypes.float8_e4m3
DRM = mybir.MatmulPerfMode.DoubleRow


def _build_program():
    """Raw Bass (no Tile): this walrus build rejects instructions carrying
    more than ~1 embedded sync wait, so all synchronization is standalone
    wait_ge instructions with manually-managed semaphores.

    Schedule (per core, ~180us):
      - The framework preamble holds every engine until ~8us; the first
        DMA cannot issue before ~7.5us and each dma_start costs ~0.6us of
        Sync issue time, so wave 1 is kept to 7 streams.
      - PE pre-warm: 11 dummy matmuls (~4.7us at the cold clock) end
        about when wave 1 lands, flipping the HAM clock gate to full
        speed with no idle gap (a >2us PE gap resets the clock).
      - No phases: tile k is a contiguous 16-matmul group gated on
        interleaved row-chunks of all four x slabs, so the PE never
        waits mid-tile and the clock never drops.
      - DVE drains each PSUM bank with a plain f32->bf16 copy; outputs
        stream out in 2-tile chunks with a split final chunk."""
    nc = bass.Bass()
    bf = mybir.dt.bfloat16
    f8 = mybir.dt.float8e4

    xa_d = nc.dram_tensor("xa", [128, FLAT], bf, kind="ExternalInput")
    x8_d = nc.dram_tensor("x8", [128, FLAT], f8, kind="ExternalInput")
    xb_d = nc.dram_tensor("xb", [128, FLAT], bf, kind="ExternalInput")
    xc_d = nc.dram_tensor("xc", [128, FLAT], bf, kind="ExternalInput")
    wt_d = nc.dram_tensor("wt", [128, 3 * NSLOT * 128], bf, kind="ExternalInput")
    w8_d = nc.dram_tensor("w8", [128, 3 * 2 * 256], f8, kind="ExternalInput")
    out_d = nc.dram_tensor("out", [128, 3 * HHALF * W], bf, kind="ExternalOutput")

    NPS = 8           # psum banks in rotation
    OCH = 4           # out-DMA granularity: blocks per chunk
    NT = 3 * NBLK     # 48 tiles
    WTM = NSLOT * 128  # wt cols per m-chunk
    # x row-chunk upper bounds (exclusive); tile k reads x rows <= 4*(k%16)+5,
    # so chunk c is first needed by tile FIRST_TILE[c]
    CH = [6, 14, 26, 38, 50, 62, 66]
    FIRST_TILE = [0, 1, 3, 6, 9, 12, 15]

    from contextlib import ExitStack
    with ExitStack() as ctx:
        xa_t = ctx.enter_context(nc.sbuf_tensor([128, FLAT], bf))
        x8_t = ctx.enter_context(nc.sbuf_tensor([128, FLAT], f8))
        xb_t = ctx.enter_context(nc.sbuf_tensor([128, FLAT], bf))
        xc_t = ctx.enter_context(nc.sbuf_tensor([128, FLAT], bf))
        wt_t = ctx.enter_context(nc.sbuf_tensor([128, 3 * NSLOT * 128], bf))
        w8_t = ctx.enter_context(nc.sbuf_tensor([128, 3 * 2 * 256], f8))
        st_t = ctx.enter_context(nc.sbuf_tensor([128, 3 * HHALF * W], bf))
        ps_t = ctx.enter_context(nc.psum_tensor([128, NPS * NFREE], mybir.dt.float32))
        dwt = ctx.enter_context(nc.semaphore("dwt"))
        dxa0 = ctx.enter_context(nc.semaphore("dxa0"))
        dxb0 = ctx.enter_context(nc.semaphore("dxb0"))
        dx = ctx.enter_context(nc.semaphore("dx"))
        dwtr = ctx.enter_context(nc.semaphore("dwtr"))
        pes = ctx.enter_context(nc.semaphore("pes"))
        dve = ctx.enter_context(nc.semaphore("dve"))
        dout = ctx.enter_context(nc.semaphore("dout"))
        warm = ctx.enter_context(nc.semaphore("warm"))
        block = ctx.enter_context(nc.Block())
        xa_v = xa_t[:].rearrange("p (r c) -> p r c", c=WP)
        xb_v = xb_t[:].rearrange("p (r c) -> p r c", c=WP)
        xc_v = xc_t[:].rearrange("p (r c) -> p r c", c=WP)

        def wt_ap(m, s):
            lo = (m * NSLOT + s) * 128
            return wt_t[:, lo:lo + 128]

        def w8_ap(m, d):
            lo = (m * 2 + d) * 256
            return w8_t[:, lo:lo + 256].rearrange("p (t q) -> p t q", t=2)

        def dr_mv(j0, j, h0=0, rsz=RB):
            # moving AP [p, t(2):WP, r(rsz):WP, c(W):1] at row j0+h0, col j
            return bass_rust.AP(
                x8_t[:].tensor, (j0 + h0) * WP + j,
                bass_rust.VecI64Pair(
                    [[FLAT, 128], [WP, 2], [WP, rsz], [1, W]]))

        QBANK = [7, 4, 5, 6]   # final-tile quarter -> psum bank

        def emit_tile_a(k, h0=0, rsz=RB, qb=None):
            # part 1: the 10 ci[0:128] slots (8 bf16 xa + 2 fp8 DR)
            m, blk = divmod(k, NBLK)
            j0 = blk * RB
            if qb is None:
                ps = ps_t[:, (k % NPS) * NFREE:(k % NPS + 1) * NFREE]
            else:
                ps = ps_t[:, qb * NFREE:qb * NFREE + rsz * W]
            for s, (kh, kw) in enumerate(TAPS_BF):
                nc.tensor.matmul(
                    ps, wt_ap(m, s),
                    xa_v[:, j0 + h0 + kh: j0 + h0 + kh + rsz, kw: kw + W],
                    start=(s == 0), stop=False)
            for d in DR_COLS:
                nc.tensor.matmul(ps, w8_ap(m, d), dr_mv(j0, d, h0=h0, rsz=rsz),
                                 start=False, stop=False, perf_mode=DRM)

        def emit_tile_b(k, h0=0, rsz=RB, qb=None):
            # part 2: the 6 ci[128:192] pair slots
            m, blk = divmod(k, NBLK)
            j0 = blk * RB
            if qb is None:
                ps = ps_t[:, (k % NPS) * NFREE:(k % NPS + 1) * NFREE]
            else:
                ps = ps_t[:, qb * NFREE:qb * NFREE + rsz * W]
            for i, (ta, _tb) in enumerate(PAIRS_XB):
                nc.tensor.matmul(
                    ps, wt_ap(m, 8 + i),
                    xb_v[:, j0 + h0 + ta[0]: j0 + h0 + ta[0] + rsz,
                         ta[1]: ta[1] + W],
                    start=False, stop=False)
            mm = nc.tensor.matmul(
                ps, wt_ap(m, 13),
                xc_v[:, j0 + h0 + PAIR_XC[0][0]: j0 + h0 + PAIR_XC[0][0] + rsz,
                     PAIR_XC[0][1]: PAIR_XC[0][1] + W],
                start=False, stop=True)
            mm.then_inc(pes, 1)

        def emit_tile(k):
            emit_tile_a(k)
            emit_tile_b(k)

        @block.sync
        def _(sync):
            def split2(dst, src, lo, hi, sem):
                mid = ((lo + hi) // 2 // 4) * 4
                sync.dma_start(dst[:, lo:mid], src[:, lo:mid]).then_inc(sem, 16)
                sync.dma_start(dst[:, mid:hi], src[:, mid:hi]).then_inc(sem, 16)

            def xchunk(c):
                lo = 0 if c == 0 else CH[c - 1] * WP
                hi = CH[c] * WP
                for dst, src in ((xa_t, xa_d), (xb_t, xb_d), (xc_t, xc_d),
                                 (x8_t, x8_d)):
                    sync.dma_start(dst[:, lo:hi], src[:, lo:hi]).then_inc(dx, 16)

            # wave 1, in issue-priority order (each dma_start costs ~0.6us
            # of issue time, so order = priority): tile 0's first 10
            # matmuls need only wt slots 0..7 (the first two thirds of the
            # m0 chunk), w8-m0, and xa/x8 rows [0,6); its last 6 matmuls
            # add the pair-slot weights (third wt chunk) and xb/xc rows.
            # Half of wave 1 issues from the otherwise-idle Scalar engine
            # (also a HWDGE) so the two issue streams run in parallel and
            # the tile-0 gate lands ~1.5us earlier.
            W3 = WTM // 3 // 4 * 4
            c0 = CH[0] * WP
            sync.dma_start(wt_t[:, 0:W3], wt_d[:, 0:W3]).then_inc(dwt, 16)
            split2(xa_t, xa_d, 0, c0, dxa0)
            sync.dma_start(wt_t[:, W3:2 * W3], wt_d[:, W3:2 * W3]).then_inc(dwt, 16)
            xchunk(1)
            # keep ~2 chunks in flight: issue chunk c+2 once chunk c landed
            sync.wait_ge(dxb0, 48)
            xchunk(2)
            for c in range(3, len(CH)):
                sync.wait_ge(dx, 64 * (c - 2))
                xchunk(c)
            # m1/m2 weights (needed from tile 16)
            split2(wt_t, wt_d, WTM, 2 * WTM, dwtr)
            split2(wt_t, wt_d, 2 * WTM, 3 * WTM, dwtr)
            sync.dma_start(w8_t[:, 512:1536], w8_d[:, 512:1536]).then_inc(dwtr, 16)
            # output: tiles 0..43 in chunks of OCH=4 (fewer, bigger DMAs
            # contend less with the PE's SBUF reads), then a tapered tail:
            # tiles 44-45 paired, 46 alone, 47 as four quarter-DMAs
            for c in range(11):
                lo, hi = c * OCH * NFREE, (c + 1) * OCH * NFREE
                sync.wait_ge(dve, OCH * (c + 1))
                sync.dma_start(out_d[:, lo:hi], st_t[:, lo:hi]).then_inc(dout, 16)
            sync.wait_ge(dve, 46)
            sync.dma_start(out_d[:, 44 * NFREE:46 * NFREE],
                           st_t[:, 44 * NFREE:46 * NFREE]).then_inc(dout, 16)
            sync.wait_ge(dve, 47)
            sync.dma_start(out_d[:, 46 * NFREE:47 * NFREE],
                           st_t[:, 46 * NFREE:47 * NFREE]).then_inc(dout, 16)
            QF = NFREE // 4
            base = 47 * NFREE
            for q in range(4):
                sync.wait_ge(dve, NT + q)
                sync.dma_start(out_d[:, base + q * QF:base + (q + 1) * QF],
                               st_t[:, base + q * QF:base + (q + 1) * QF]
                               ).then_inc(dout, 16)
            sync.wait_ge(dout, 16 * 17)

        @block.scalar
        def _(scalar):
            # wave-1 second issue stream, parallel to Sync's
            scalar.dma_start(x8_t[:, 0:CH[0] * WP],
                             x8_d[:, 0:CH[0] * WP]).then_inc(dxa0, 16)
            scalar.dma_start(w8_t[:, 0:512], w8_d[:, 0:512]).then_inc(dwt, 16)
            scalar.dma_start(wt_t[:, 2 * (NSLOT * 128 // 3 // 4 * 4):NSLOT * 128],
                             wt_d[:, 2 * (NSLOT * 128 // 3 // 4 * 4):NSLOT * 128]
                             ).then_inc(dxb0, 16)
            scalar.dma_start(xb_t[:, 0:CH[0] * WP],
                             xb_d[:, 0:CH[0] * WP]).then_inc(dxb0, 16)
            scalar.dma_start(xc_t[:, 0:CH[0] * WP],
                             xc_d[:, 0:CH[0] * WP]).then_inc(dxb0, 16)

        @block.tensor
        def _(tensor):
            # pre-warm the PE HAM clock gate during the initial DMA wait.
            # The dummies must be FULL-K (128 partitions): K=1 dummies draw
            # no array power and never flip the clock -- the first ~7
            # full-K matmuls run at ~427ns (low pstate), then ~216ns. 17
            # of them end about when wave 1 lands; any idle gap >~1us
            # drops the clock again. st_t garbage is fine: bank 7's first
            # real user (tile 7) starts its group with start=True.
            tensor.wait_ge(warm, 1)
            for _ in range(13):
                nc.tensor.matmul(
                    ps_t[:, 7 * NFREE:8 * NFREE],
                    st_t[:, 0:128],
                    st_t[:, 0:NFREE],
                    start=True,
                    stop=True,
                )
            # tile 0 split-phase: its first 10 matmuls need only wt/w8 +
            # xa/x8 rows [0,6), which are DMA-issued ahead of xb/xc -- the
            # PE starts ~1.5us earlier and the clock never drops.
            tensor.wait_ge(dwt, 48)
            tensor.wait_ge(dxa0, 48)
            emit_tile_a(0)
            tensor.wait_ge(dxb0, 48)
            emit_tile_b(0)
            gates = {t: 64 * c for c, t in enumerate(FIRST_TILE) if c > 0}
            for k in range(1, NT - 1):
                if k in gates:
                    tensor.wait_ge(dx, gates[k])
                if k == 16:
                    tensor.wait_ge(dwtr, 80)
                # one bank-reuse wait covers 4 tiles: tiles k..k+3 need at
                # most dve >= k+3-(NPS-1), and DVE lags PE by well under
                # the 3-tile slack this leaves.
                if k >= NPS and (k - NPS) % 4 == 0:
                    tensor.wait_ge(dve, min(k + 3, NT - 1) - NPS + 1)
                emit_tile(k)
            # final tile split into four 1-row quarters in four different
            # banks: each quarter's epilogue+out-DMA overlaps the next
            # quarter's matmuls, and the last DMA is only 32KB
            for q in range(4):
                # QBANK[q] is free once its previous tile's copy landed:
                # bank 7 <- tile 39 (dve>=40, covered by the k=44 wait),
                # bank 4/5/6 <- tiles 44/45/46 (dve >= 45/46/47)
                if q > 0:
                    tensor.wait_ge(dve, NT - 4 + q)
                emit_tile_a(NT - 1, h0=q, rsz=1, qb=QBANK[q])
                emit_tile_b(NT - 1, h0=q, rsz=1, qb=QBANK[q])

        @block.vector
        def _(vector):
            nc.vector.memset(st_t[0:1, 0:NFREE], 0.0).then_inc(warm, 1)
            for k in range(NT - 1):
                ps = ps_t[:, (k % NPS) * NFREE:(k % NPS + 1) * NFREE]
                vector.wait_ge(pes, k + 1)
                nc.vector.tensor_copy(
                    st_t[:, k * NFREE:(k + 1) * NFREE], ps).then_inc(dve, 1)
            # final tile: four quarter-width copies matching the split groups
            k = NT - 1
            QF = NFREE // 4
            for q in range(4):
                ps_q = ps_t[:, QBANK[q] * NFREE:QBANK[q] * NFREE + QF]
                vector.wait_ge(pes, k + 1 + q)
                nc.vector.tensor_copy(
                    st_t[:, k * NFREE + q * QF:k * NFREE + (q + 1) * QF],
                    ps_q).then_inc(dve, 1)
    return nc


def _causal_mask():
    m = np.ones((KH, KW), dtype=np.float32)
    m[KH // 2, KW // 2:] = 0.0
    m[KH // 2 + 1:, :] = 0.0
    return m


def _prepare_in_maps(x, weight, bias, mask):
    # window-any of mask -> valid [B, H, W]
    ind = (np.asarray(mask)[:, 0] != 0)
    indp = np.zeros((B, H + 2 * PAD, W + 2 * PAD), dtype=bool)
    indp[:, PAD:PAD + H, PAD:PAD + W] = ind
    valid = np.zeros((B, H, W), dtype=bool)
    for dh in range(KH):
        for dw in range(KW):
            valid |= indp[:, dh:dh + H, dw:dw + W]

    w_sc = np.asarray(weight, dtype=np.float32) * _causal_mask()[None, None] * WSCALE
    w_bf = w_sc.astype(BF16)
    w_f8 = w_sc.astype(FP8)

    # bf16 weights: m-major image [128 K, 3 m-chunks x 14 slots x 128 couts]
    wt = np.zeros((3, NSLOT, 128, 128), dtype=BF16)
    for m in range(3):
        cl, ch = m * 128, (m + 1) * 128
        for s, (kh, kw) in enumerate(TAPS_BF):
            wt[m, s] = w_bf[cl:ch, 0:128, kh, kw].T
        for i, (ta, tb) in enumerate(PAIRS_XB):
            wt[m, 8 + i, 0:64] = w_bf[cl:ch, 128:192, ta[0], ta[1]].T
            wt[m, 8 + i, 64:128] = w_bf[cl:ch, 128:192, tb[0], tb[1]].T
        ta, tb = PAIR_XC
        wt[m, 13, 0:64] = w_bf[cl:ch, 128:192, ta[0], ta[1]].T
        wt[m, 13, 64:128] = w_bf[cl:ch, 128:192, tb[0], tb[1]].T
    wt_sb = np.ascontiguousarray(wt.transpose(2, 0, 1, 3)).reshape(128, 3 * NSLOT * 128)

    # fp8 DR weights: [128 K, 3 m x 2 slots x 2 t x 128 couts], t = kh
    w8 = np.zeros((3, 2, 2, 128, 128), dtype=FP8)
    for m in range(3):
        cl, ch = m * 128, (m + 1) * 128
        for d in DR_COLS:
            for t in range(2):
                w8[m, d, t] = w_f8[cl:ch, 0:128, t, d].T
    w8_sb = np.ascontiguousarray(w8.transpose(3, 0, 1, 2, 4)).reshape(128, 3 * 2 * 256)

    x_f32 = np.asarray(x, dtype=np.float32)
    x_bf = x_f32.astype(BF16)
    x_f8 = x_f32.astype(FP8)

    in_maps = []
    for c in range(NCORES):
        b, half = c // 2, c % 2
        r0 = half * HHALF
        lo = r0 - PAD
        src_lo = max(lo, 0)

        xp = np.zeros((128, NROWS, WP), dtype=BF16)
        xp[:, src_lo - lo:, PAD:PAD + W] = x_bf[b, 0:128, src_lo:r0 + HHALF, :]
        xp8 = np.zeros((128, NROWS, WP), dtype=FP8)
        xp8[:, src_lo - lo:, PAD:PAD + W] = x_f8[b, 0:128, src_lo:r0 + HHALF, :]

        x2p = np.zeros((64, NROWS, WP), dtype=BF16)
        x2p[:, src_lo - lo:, PAD:PAD + W] = x_bf[b, 128:192, src_lo:r0 + HHALF, :]
        x2 = x2p.reshape(64, FLAT)
        sh1 = np.zeros_like(x2)
        sh1[:, :-1] = x2[:, 1:]
        shr = np.zeros_like(x2)
        shr[:, :-WP] = x2[:, WP:]
        in_maps.append({
            "xa": np.ascontiguousarray(xp.reshape(128, FLAT)),
            "x8": np.ascontiguousarray(xp8.reshape(128, FLAT)),
            "xb": np.ascontiguousarray(np.concatenate([x2, sh1], axis=0)),
            "xc": np.ascontiguousarray(np.concatenate([x2, shr], axis=0)),
            "wt": wt_sb,
            "w8": w8_sb,
        })
    return in_maps, valid


def _assemble(results, valid, bias):
    bias_f = np.asarray(bias, dtype=np.float32)
    out_full = np.empty((B, COUT, H, W), dtype=np.float32)
    inv = np.float32(1.0 / WSCALE)
    for c in range(NCORES):
        b, half = c // 2, c % 2
        o = np.asarray(results[c]["out"]).astype(np.float32)
        o4 = o.reshape(128, 3, HHALF, W).transpose(1, 0, 2, 3).reshape(COUT, HHALF, W)
        o4 = o4 * inv + bias_f[:, None, None]
        v = valid[b, half * HHALF:(half + 1) * HHALF, :]
        out_full[b, :, half * HHALF:(half + 1) * HHALF, :] = np.where(v[None], o4, 0.0)
    return out_full


def kernel(x, weight, bias, mask, _trace=False):
    in_maps, valid = _prepare_in_maps(x, weight, bias, mask)
    nc = _build_program()
    res = run_bass_kernel_spmd(nc, in_maps, core_ids=list(range(NCORES)),
                               trace=_trace)
    out = _assemble(res.results, valid, bias)
    if _trace:
        return out, res
    return out
